# revision 1
# baseline (speedup 1.0000x reference)
"""VQ codebook lookup kernel for Trainium2 (8 NeuronCores, data-parallel).

Computes out[b] = values[argmin_k ||x[b] - keys[k]||] for
x [65536, 512], keys/values [1024, 512] fp32.

Strategy (per core, batch shard of 8192 rows):
  - Host prep: transpose x shard to [512, 8192]; pass (2*keys)^T [512, 1024]
    and |keys|^2 replicated [128, 1024].  argmin of distance == argmax of
    s = 2*x.k - |k|^2, which the device computes as matmul + bias-subtract.
  - Device: PE matmul (fp32) -> DVE subtract -> DVE max/max_index ->
    indirect-DMA gather of values rows -> DMA out.
"""

import numpy as np

_B = 65536
_D = 512
_K = 1024
_NCORES = 8
_BL = _B // _NCORES  # 8192 rows per core
_P = 128
_BBLK = 512          # b columns loaded per DMA
_BT = 128            # b rows per matmul tile (PSUM partition dim)
_DC = _D // _P       # 4 contraction chunks

_cached = None


def _build():
    import concourse.mybir as mybir
    from concourse import bacc
    from concourse.bass import IndirectOffsetOnAxis
    from concourse.tile import TileContext

    f32 = mybir.dt.float32
    u32 = mybir.dt.uint32

    nc = bacc.Bacc("TRN2", target_bir_lowering=False, debug=False,
                   num_devices=_NCORES)
    xT = nc.dram_tensor("xT", [_D, _BL], f32, kind="ExternalInput")
    kT2 = nc.dram_tensor("kT2", [_D, _K], f32, kind="ExternalInput")
    k2r = nc.dram_tensor("k2r", [_P, _K], f32, kind="ExternalInput")
    vals = nc.dram_tensor("vals", [_K, _D], f32, kind="ExternalInput")
    out = nc.dram_tensor("out", [_BL, _D], f32, kind="ExternalOutput")

    xT3 = xT.rearrange("(do p) b -> p do b", p=_P)     # [128, 4, 8192]
    kT3 = kT2.rearrange("(do p) k -> p do k", p=_P)    # [128, 4, 1024]

    with TileContext(nc) as tc:
        with (
            tc.tile_pool(name="const", bufs=1) as cpool,
            tc.tile_pool(name="xp", bufs=3) as xpool,
            tc.tile_pool(name="sp", bufs=3) as spool,
            tc.tile_pool(name="st", bufs=4) as stpool,
            tc.tile_pool(name="gp", bufs=4) as gpool,
            tc.tile_pool(name="ps", bufs=4, space="PSUM") as pspool,
        ):
            keys_sb = cpool.tile([_P, _DC, _K], f32)
            nc.sync.dma_start(keys_sb[:], kT3[:, :, :])
            k2_sb = cpool.tile([_P, _K], f32)
            nc.sync.dma_start(k2_sb[:], k2r[:, :])

            for blk in range(_BL // _BBLK):
                xt = xpool.tile([_P, _DC, _BBLK], f32)
                nc.sync.dma_start(
                    xt[:], xT3[:, :, blk * _BBLK:(blk + 1) * _BBLK])

                for sub in range(_BBLK // _BT):
                    bt = blk * (_BBLK // _BT) + sub
                    s = spool.tile([_P, _K], f32)
                    for h in range(2):
                        ps = pspool.tile([_P, 512], f32)
                        for dc in range(_DC):
                            nc.tensor.matmul(
                                ps[:],
                                lhsT=xt[:, dc, sub * _BT:(sub + 1) * _BT],
                                rhs=keys_sb[:, dc, h * 512:(h + 1) * 512],
                                start=(dc == 0),
                                stop=(dc == _DC - 1),
                            )
                        # s = 2*x.k - |k|^2 (fused PSUM->SBUF copy)
                        nc.vector.tensor_sub(
                            out=s[:, h * 512:(h + 1) * 512],
                            in0=ps[:],
                            in1=k2_sb[:, h * 512:(h + 1) * 512],
                        )
                    mx = stpool.tile([_P, 8], f32)
                    nc.vector.max(out=mx[:], in_=s[:])
                    idx = stpool.tile([_P, 8], u32)
                    nc.vector.max_index(out=idx[:], in_max=mx[:], in_values=s[:])

                    g = gpool.tile([_P, _D], f32)
                    nc.gpsimd.indirect_dma_start(
                        out=g[:],
                        out_offset=None,
                        in_=vals[:, :],
                        in_offset=IndirectOffsetOnAxis(ap=idx[:, :1], axis=0),
                    )
                    nc.sync.dma_start(out[bt * _BT:(bt + 1) * _BT, :], g[:])

    nc.compile()
    return nc


def _get_nc():
    global _cached
    if _cached is None:
        _cached = _build()
    return _cached


def kernel(x, keys, values):
    from concourse.bass_utils import run_bass_kernel_spmd

    nc = _get_nc()

    x = np.asarray(x, dtype=np.float32)
    keys = np.asarray(keys, dtype=np.float32)
    values = np.asarray(values, dtype=np.float32)

    kT2 = np.ascontiguousarray((2.0 * keys).T)                  # [512, 1024]
    k2 = np.einsum("kd,kd->k", keys, keys).astype(np.float32)   # [1024]
    k2r = np.ascontiguousarray(np.broadcast_to(k2, (_P, _K)))   # [128, 1024]

    in_maps = []
    for c in range(_NCORES):
        xs = np.ascontiguousarray(x[c * _BL:(c + 1) * _BL].T)   # [512, 8192]
        in_maps.append({"xT": xs, "kT2": kT2, "k2r": k2r, "vals": values})

    res = run_bass_kernel_spmd(nc, in_maps, core_ids=list(range(_NCORES)))
    return np.concatenate([r["out"] for r in res.results], axis=0)


# revision 2
# speedup vs baseline: 1.4587x; 1.4587x over previous
"""VQ codebook lookup kernel for Trainium2 (8 NeuronCores, data-parallel).

Computes out[b] = values[argmin_k ||x[b] - keys[k]||] for
x [65536, 512], keys/values [1024, 512] fp32.

Strategy (per core, batch shard of 8192 rows):
  - argmin of distance == argmax of s = 2*x.k - |k|^2.
  - fp32 precision via bf16 hi/lo split, 3 matmul passes
    (hi*hi + hi*lo + lo*hi); the -|k|^2 bias is folded into the same
    PSUM accumulation as a 4th matmul against an all-ones stationary
    operand (bias rows = 3-way bf16 split of -|k|^2).
  - Host prep: transpose x shard to [512, 8192] and split to bf16 hi/lo;
    same for (2*keys)^T.
  - Device: PE matmuls -> DVE MAX8/FIND_INDEX8 straight from PSUM ->
    indirect-DMA gather of values rows -> DMA out.
"""

import numpy as np

_B = 65536
_D = 512
_K = 1024
_NCORES = 8
_BL = _B // _NCORES  # 8192 rows per core
_P = 128
_BBLK = 512          # b columns loaded per DMA
_BT = 128            # b rows per matmul tile (PSUM partition dim)
_DC = _D // _P       # 4 contraction chunks

_cached = None


def _build():
    import concourse.mybir as mybir
    from concourse import bacc
    from concourse.bass import IndirectOffsetOnAxis
    from concourse.tile import TileContext

    f32 = mybir.dt.float32
    bf16 = mybir.dt.bfloat16
    u32 = mybir.dt.uint32

    nc = bacc.Bacc("TRN2", target_bir_lowering=False, debug=False,
                   num_devices=_NCORES)
    xTh = nc.dram_tensor("xTh", [_D, _BL], bf16, kind="ExternalInput")
    xTl = nc.dram_tensor("xTl", [_D, _BL], bf16, kind="ExternalInput")
    kTh = nc.dram_tensor("kTh", [_D, _K], bf16, kind="ExternalInput")
    kTl = nc.dram_tensor("kTl", [_D, _K], bf16, kind="ExternalInput")
    biasp = nc.dram_tensor("biasp", [_P, _K], bf16, kind="ExternalInput")
    vals = nc.dram_tensor("vals", [_K, _D], f32, kind="ExternalInput")
    out = nc.dram_tensor("out", [_BL, _D], f32, kind="ExternalOutput")

    xTh3 = xTh.rearrange("(do p) b -> p do b", p=_P)   # [128, 4, 8192]
    xTl3 = xTl.rearrange("(do p) b -> p do b", p=_P)
    kTh3 = kTh.rearrange("(do p) k -> p do k", p=_P)   # [128, 4, 1024]
    kTl3 = kTl.rearrange("(do p) k -> p do k", p=_P)

    with TileContext(nc) as tc:
        with (
            tc.tile_pool(name="const", bufs=1) as cpool,
            tc.tile_pool(name="xp", bufs=3) as xpool,
            tc.tile_pool(name="st", bufs=4) as stpool,
            tc.tile_pool(name="gp", bufs=4) as gpool,
            tc.tile_pool(name="ps", bufs=3, space="PSUM") as pspool,
        ):
            kh_sb = cpool.tile([_P, _DC, _K], bf16)
            nc.sync.dma_start(kh_sb[:], kTh3[:, :, :])
            kl_sb = cpool.tile([_P, _DC, _K], bf16)
            nc.sync.dma_start(kl_sb[:], kTl3[:, :, :])
            bias_sb = cpool.tile([_P, _K], bf16)
            nc.sync.dma_start(bias_sb[:], biasp[:, :])
            ones_sb = cpool.tile([_P, _P], bf16)
            nc.vector.memset(ones_sb[:], 1.0)

            for blk in range(_BL // _BBLK):
                xth = xpool.tile([_P, _DC, _BBLK], bf16)
                nc.sync.dma_start(
                    xth[:], xTh3[:, :, blk * _BBLK:(blk + 1) * _BBLK])
                xtl = xpool.tile([_P, _DC, _BBLK], bf16)
                nc.sync.dma_start(
                    xtl[:], xTl3[:, :, blk * _BBLK:(blk + 1) * _BBLK])

                for sub in range(_BBLK // _BT):
                    bt = blk * (_BBLK // _BT) + sub
                    bsl = slice(sub * _BT, (sub + 1) * _BT)
                    ps = pspool.tile([_P, _K], f32)
                    for h in range(2):
                        hsl = slice(h * 512, (h + 1) * 512)
                        po = ps[:, hsl]
                        nc.tensor.matmul(po, lhsT=ones_sb[:],
                                         rhs=bias_sb[:, hsl],
                                         start=True, stop=False)
                        for dc in range(_DC):
                            nc.tensor.matmul(po, lhsT=xth[:, dc, bsl],
                                             rhs=kh_sb[:, dc, hsl],
                                             start=False, stop=False)
                            nc.tensor.matmul(po, lhsT=xth[:, dc, bsl],
                                             rhs=kl_sb[:, dc, hsl],
                                             start=False, stop=False)
                        for dc in range(_DC):
                            nc.tensor.matmul(po, lhsT=xtl[:, dc, bsl],
                                             rhs=kh_sb[:, dc, hsl],
                                             start=False, stop=(dc == _DC - 1))
                    mx = stpool.tile([_P, 8], f32)
                    nc.vector.max(out=mx[:], in_=ps[:])
                    idx = stpool.tile([_P, 8], u32)
                    nc.vector.max_index(out=idx[:], in_max=mx[:], in_values=ps[:])

                    g = gpool.tile([_P, _D], f32)
                    nc.gpsimd.indirect_dma_start(
                        out=g[:],
                        out_offset=None,
                        in_=vals[:, :],
                        in_offset=IndirectOffsetOnAxis(ap=idx[:, :1], axis=0),
                    )
                    nc.sync.dma_start(out[bt * _BT:(bt + 1) * _BT, :], g[:])

    nc.compile()
    return nc


def _get_nc():
    global _cached
    if _cached is None:
        _cached = _build()
    return _cached


def _hi_lo(a):
    """Split fp32 array into bf16 hi + bf16 lo with hi + lo ~ a."""
    import ml_dtypes

    hi = a.astype(ml_dtypes.bfloat16)
    lo = (a - hi.astype(np.float32)).astype(ml_dtypes.bfloat16)
    return hi, lo


def _prepare_in_maps(x, keys, values):
    x = np.asarray(x, dtype=np.float32)
    keys = np.asarray(keys, dtype=np.float32)
    values = np.asarray(values, dtype=np.float32)

    import ml_dtypes

    k2T = np.ascontiguousarray((2.0 * keys).T)          # [512, 1024] f32
    kTh, kTl = _hi_lo(k2T)

    # 3-way bf16 split of -|k|^2, padded to 128 partition rows
    b64 = -np.einsum("kd,kd->k", keys.astype(np.float64),
                     keys.astype(np.float64))
    bh = b64.astype(ml_dtypes.bfloat16)
    bm = (b64 - bh.astype(np.float64)).astype(ml_dtypes.bfloat16)
    bl = (b64 - bh.astype(np.float64) - bm.astype(np.float64)).astype(
        ml_dtypes.bfloat16)
    biasp = np.zeros((_P, _K), dtype=ml_dtypes.bfloat16)
    biasp[0], biasp[1], biasp[2] = bh, bm, bl

    in_maps = []
    for c in range(_NCORES):
        xs = np.ascontiguousarray(x[c * _BL:(c + 1) * _BL].T)  # [512, 8192]
        xh, xl = _hi_lo(xs)
        in_maps.append({"xTh": xh, "xTl": xl, "kTh": kTh, "kTl": kTl,
                        "biasp": biasp, "vals": values})
    return in_maps


def kernel(x, keys, values):
    from concourse.bass_utils import run_bass_kernel_spmd

    nc = _get_nc()
    in_maps = _prepare_in_maps(x, keys, values)
    res = run_bass_kernel_spmd(nc, in_maps, core_ids=list(range(_NCORES)))
    return np.concatenate([r["out"] for r in res.results], axis=0)


# revision 6
# speedup vs baseline: 1.5414x; 1.0567x over previous
"""VQ codebook lookup kernel for Trainium2 (8 NeuronCores, data-parallel).

Computes out[b] = values[argmin_k ||x[b] - keys[k]||] for
x [65536, 512], keys/values [1024, 512] fp32.

Strategy (per core, batch shard of 8192 rows):
  - argmin of distance == argmax of s = 2*x.k - |k|^2.
  - fp32 precision via bf16 hi/lo split, 3 matmul passes
    (hi*hi + hi*lo + lo*hi); the -|k|^2 bias is folded into the same
    PSUM accumulation as a 4th matmul against an all-ones stationary
    operand (bias rows = 3-way bf16 split of -|k|^2).
  - Host prep: transpose x shard to [512, 8192] and split to bf16 hi/lo;
    same for (2*keys)^T.
  - Device: PE matmuls -> DVE MAX8/FIND_INDEX8 straight from PSUM ->
    indirect-DMA gather of values rows -> DMA out.
"""

import numpy as np

_B = 65536
_D = 512
_K = 1024
_NCORES = 8
_BL = _B // _NCORES  # 8192 rows per core
_P = 128
_BBLK = 512          # b columns loaded per DMA
_BT = 128            # b rows per matmul tile (PSUM partition dim)
_DC = _D // _P       # 4 contraction chunks

_cached = None

# If True, fold -|k|^2 into the PE matmul accumulation (costs 2 extra
# N=512 matmuls per b-tile).  If False, subtract it on the vector engine
# fused with the PSUM->SBUF copy.
_BIAS_ON_PE = False


def _build():
    import concourse.mybir as mybir
    from concourse import bacc
    from concourse.bass import IndirectOffsetOnAxis
    from concourse.tile import TileContext

    f32 = mybir.dt.float32
    bf16 = mybir.dt.bfloat16
    u32 = mybir.dt.uint32

    nc = bacc.Bacc("TRN2", target_bir_lowering=False, debug=False,
                   num_devices=_NCORES)
    xTh = nc.dram_tensor("xTh", [_D, _BL], bf16, kind="ExternalInput")
    xTl = nc.dram_tensor("xTl", [_D, _BL], bf16, kind="ExternalInput")
    kTh = nc.dram_tensor("kTh", [_D, _K], bf16, kind="ExternalInput")
    kTl = nc.dram_tensor("kTl", [_D, _K], bf16, kind="ExternalInput")
    biasp = nc.dram_tensor("biasp", [_P, _K], bf16, kind="ExternalInput")
    k2r = nc.dram_tensor("k2r", [_P, _K], f32, kind="ExternalInput")
    vals = nc.dram_tensor("vals", [_K, _D], f32, kind="ExternalInput")
    out = nc.dram_tensor("out", [_BL, _D], f32, kind="ExternalOutput")

    xTh3 = xTh.rearrange("(do p) b -> p do b", p=_P)   # [128, 4, 8192]
    xTl3 = xTl.rearrange("(do p) b -> p do b", p=_P)
    kTh3 = kTh.rearrange("(do p) k -> p do k", p=_P)   # [128, 4, 1024]
    kTl3 = kTl.rearrange("(do p) k -> p do k", p=_P)

    with TileContext(nc) as tc:
        with (
            tc.tile_pool(name="const", bufs=1) as cpool,
            tc.tile_pool(name="xp", bufs=3) as xpool,
            tc.tile_pool(name="sp", bufs=3) as spool,
            tc.tile_pool(name="st", bufs=4) as stpool,
            tc.tile_pool(name="gp", bufs=4) as gpool,
            tc.tile_pool(name="ps", bufs=3, space="PSUM") as pspool,
        ):
            # Split const loads by code-half so the first matmuls only wait
            # for the half-0 slices.
            kh_sb = cpool.tile([_P, _DC, _K], bf16)
            kl_sb = cpool.tile([_P, _DC, _K], bf16)
            for h in range(2):
                hsl = slice(h * 512, (h + 1) * 512)
                nc.sync.dma_start(kh_sb[:, :, hsl], kTh3[:, :, hsl])
                nc.sync.dma_start(kl_sb[:, :, hsl], kTl3[:, :, hsl])
            if _BIAS_ON_PE:
                bias_sb = cpool.tile([_P, _K], bf16)
                nc.sync.dma_start(bias_sb[:], biasp[:, :])
                ones_sb = cpool.tile([_P, _P], bf16)
                nc.vector.memset(ones_sb[:], 1.0)
            else:
                k2_sb = cpool.tile([_P, _K], f32)
                nc.sync.dma_start(k2_sb[:], k2r[:, :])

            for blk in range(_BL // _BBLK):
                xth = xpool.tile([_P, _DC, _BBLK], bf16)
                nc.sync.dma_start(
                    xth[:], xTh3[:, :, blk * _BBLK:(blk + 1) * _BBLK])
                xtl = xpool.tile([_P, _DC, _BBLK], bf16)
                nc.sync.dma_start(
                    xtl[:], xTl3[:, :, blk * _BBLK:(blk + 1) * _BBLK])

                for sub in range(_BBLK // _BT):
                    bt = blk * (_BBLK // _BT) + sub
                    bsl = slice(sub * _BT, (sub + 1) * _BT)
                    ps = pspool.tile([_P, _K], f32)
                    if not _BIAS_ON_PE:
                        s = spool.tile([_P, _K], f32)
                    for h in range(2):
                        hsl = slice(h * 512, (h + 1) * 512)
                        po = ps[:, hsl]
                        if _BIAS_ON_PE:
                            nc.tensor.matmul(po, lhsT=ones_sb[:],
                                             rhs=bias_sb[:, hsl],
                                             start=True, stop=False)
                        for dc in range(_DC):
                            nc.tensor.matmul(po, lhsT=xth[:, dc, bsl],
                                             rhs=kh_sb[:, dc, hsl],
                                             start=(not _BIAS_ON_PE and dc == 0),
                                             stop=False)
                            nc.tensor.matmul(po, lhsT=xth[:, dc, bsl],
                                             rhs=kl_sb[:, dc, hsl],
                                             start=False, stop=False)
                        for dc in range(_DC):
                            nc.tensor.matmul(po, lhsT=xtl[:, dc, bsl],
                                             rhs=kh_sb[:, dc, hsl],
                                             start=False, stop=(dc == _DC - 1))
                        if not _BIAS_ON_PE:
                            # s = 2*x.k - |k|^2, fused PSUM->SBUF move
                            nc.vector.tensor_sub(
                                out=s[:, hsl], in0=po, in1=k2_sb[:, hsl])
                    sc = ps if _BIAS_ON_PE else s
                    mx = stpool.tile([_P, 8], f32)
                    nc.vector.max(out=mx[:], in_=sc[:])
                    idx = stpool.tile([_P, 8], u32)
                    nc.vector.max_index(out=idx[:], in_max=mx[:], in_values=sc[:])

                    g = gpool.tile([_P, _D], f32)
                    nc.gpsimd.indirect_dma_start(
                        out=g[:],
                        out_offset=None,
                        in_=vals[:, :],
                        in_offset=IndirectOffsetOnAxis(ap=idx[:, :1], axis=0),
                    )
                    nc.sync.dma_start(out[bt * _BT:(bt + 1) * _BT, :], g[:])

    nc.compile()
    return nc


def _get_nc():
    global _cached
    if _cached is None:
        _cached = _build()
    return _cached


def _hi_lo(a):
    """Split fp32 array into bf16 hi + bf16 lo with hi + lo ~ a."""
    import ml_dtypes

    hi = a.astype(ml_dtypes.bfloat16)
    lo = (a - hi.astype(np.float32)).astype(ml_dtypes.bfloat16)
    return hi, lo


def _prepare_in_maps(x, keys, values):
    x = np.asarray(x, dtype=np.float32)
    keys = np.asarray(keys, dtype=np.float32)
    values = np.asarray(values, dtype=np.float32)

    import ml_dtypes

    k2T = np.ascontiguousarray((2.0 * keys).T)          # [512, 1024] f32
    kTh, kTl = _hi_lo(k2T)

    # 3-way bf16 split of -|k|^2, padded to 128 partition rows
    b64 = -np.einsum("kd,kd->k", keys.astype(np.float64),
                     keys.astype(np.float64))
    bh = b64.astype(ml_dtypes.bfloat16)
    bm = (b64 - bh.astype(np.float64)).astype(ml_dtypes.bfloat16)
    bl = (b64 - bh.astype(np.float64) - bm.astype(np.float64)).astype(
        ml_dtypes.bfloat16)
    biasp = np.zeros((_P, _K), dtype=ml_dtypes.bfloat16)
    biasp[0], biasp[1], biasp[2] = bh, bm, bl

    k2 = np.einsum("kd,kd->k", keys, keys).astype(np.float32)
    k2r = np.ascontiguousarray(np.broadcast_to(k2, (_P, _K)))

    in_maps = []
    for c in range(_NCORES):
        xs = np.ascontiguousarray(x[c * _BL:(c + 1) * _BL].T)  # [512, 8192]
        xh, xl = _hi_lo(xs)
        in_maps.append({"xTh": xh, "xTl": xl, "kTh": kTh, "kTl": kTl,
                        "biasp": biasp, "k2r": k2r, "vals": values})
    return in_maps


def kernel(x, keys, values):
    from concourse.bass_utils import run_bass_kernel_spmd

    nc = _get_nc()
    in_maps = _prepare_in_maps(x, keys, values)
    res = run_bass_kernel_spmd(nc, in_maps, core_ids=list(range(_NCORES)))
    return np.concatenate([r["out"] for r in res.results], axis=0)


# revision 7
# speedup vs baseline: 1.5564x; 1.0097x over previous
"""VQ codebook lookup kernel for Trainium2 (8 NeuronCores, data-parallel).

Computes out[b] = values[argmin_k ||x[b] - keys[k]||] for
x [65536, 512], keys/values [1024, 512] fp32.

Strategy (per core, batch shard of 8192 rows):
  - argmin of distance == argmax of s = 2*x.k - |k|^2.
  - fp32 precision via bf16 hi/lo split, 3 matmul passes
    (hi*hi + hi*lo + lo*hi); the -|k|^2 bias is folded into the same
    PSUM accumulation as a 4th matmul against an all-ones stationary
    operand (bias rows = 3-way bf16 split of -|k|^2).
  - Host prep: transpose x shard to [512, 8192] and split to bf16 hi/lo;
    same for (2*keys)^T.
  - Device: PE matmuls -> DVE MAX8/FIND_INDEX8 straight from PSUM ->
    indirect-DMA gather of values rows -> DMA out.
"""

import numpy as np

_B = 65536
_D = 512
_K = 1024
_NCORES = 8
_BL = _B // _NCORES  # 8192 rows per core
_P = 128
_BBLK = 512          # b columns loaded per DMA
_BT = 128            # b rows per matmul tile (PSUM partition dim)
_DC = _D // _P       # 4 contraction chunks

_cached = None

# If True, fold -|k|^2 into the PE matmul accumulation (costs 2 extra
# N=512 matmuls per b-tile).  If False, subtract it on the vector engine
# fused with the PSUM->SBUF copy.
_BIAS_ON_PE = False


def _build():
    import concourse.mybir as mybir
    from concourse import bacc
    from concourse.bass import IndirectOffsetOnAxis
    from concourse.tile import TileContext

    f32 = mybir.dt.float32
    bf16 = mybir.dt.bfloat16
    u32 = mybir.dt.uint32

    nc = bacc.Bacc("TRN2", target_bir_lowering=False, debug=False,
                   num_devices=_NCORES)
    xTh = nc.dram_tensor("xTh", [_D, _BL], bf16, kind="ExternalInput")
    xTl = nc.dram_tensor("xTl", [_D, _BL], bf16, kind="ExternalInput")
    kTh = nc.dram_tensor("kTh", [_D, _K], bf16, kind="ExternalInput")
    kTl = nc.dram_tensor("kTl", [_D, _K], bf16, kind="ExternalInput")
    biasp = nc.dram_tensor("biasp", [_P, _K], bf16, kind="ExternalInput")
    k2r = nc.dram_tensor("k2r", [_P, _K], f32, kind="ExternalInput")
    vals = nc.dram_tensor("vals", [_K, _D], f32, kind="ExternalInput")
    out = nc.dram_tensor("out", [_BL, _D], f32, kind="ExternalOutput")

    xTh3 = xTh.rearrange("(do p) b -> p do b", p=_P)   # [128, 4, 8192]
    xTl3 = xTl.rearrange("(do p) b -> p do b", p=_P)
    kTh3 = kTh.rearrange("(do p) k -> p do k", p=_P)   # [128, 4, 1024]
    kTl3 = kTl.rearrange("(do p) k -> p do k", p=_P)

    with TileContext(nc) as tc:
        with (
            tc.tile_pool(name="const", bufs=1) as cpool,
            tc.tile_pool(name="xp", bufs=3) as xpool,
            tc.tile_pool(name="sp", bufs=3) as spool,
            tc.tile_pool(name="st", bufs=4) as stpool,
            tc.tile_pool(name="gp", bufs=4) as gpool,
            tc.tile_pool(name="ps", bufs=3, space="PSUM") as pspool,
        ):
    # Const loads go on the Scalar engine's HWDGE queue so they overlap
    # with the x-block loads issued from the Sync engine (descriptor
    # generation serializes per issuing engine).
            kh_sb = cpool.tile([_P, _DC, _K], bf16)
            nc.scalar.dma_start(kh_sb[:], kTh3[:, :, :])
            kl_sb = cpool.tile([_P, _DC, _K], bf16)
            nc.scalar.dma_start(kl_sb[:], kTl3[:, :, :])
            if _BIAS_ON_PE:
                bias_sb = cpool.tile([_P, _K], bf16)
                nc.scalar.dma_start(bias_sb[:], biasp[:, :])
                ones_sb = cpool.tile([_P, _P], bf16)
                nc.vector.memset(ones_sb[:], 1.0)
            else:
                k2_sb = cpool.tile([_P, _K], f32)
                nc.scalar.dma_start(k2_sb[:], k2r[:, :])

            for blk in range(_BL // _BBLK):
                xth = xpool.tile([_P, _DC, _BBLK], bf16)
                nc.sync.dma_start(
                    xth[:], xTh3[:, :, blk * _BBLK:(blk + 1) * _BBLK])
                xtl = xpool.tile([_P, _DC, _BBLK], bf16)
                nc.sync.dma_start(
                    xtl[:], xTl3[:, :, blk * _BBLK:(blk + 1) * _BBLK])

                for sub in range(_BBLK // _BT):
                    bt = blk * (_BBLK // _BT) + sub
                    bsl = slice(sub * _BT, (sub + 1) * _BT)
                    ps = pspool.tile([_P, _K], f32)
                    if not _BIAS_ON_PE:
                        s = spool.tile([_P, _K], f32)
                    for h in range(2):
                        hsl = slice(h * 512, (h + 1) * 512)
                        po = ps[:, hsl]
                        if _BIAS_ON_PE:
                            nc.tensor.matmul(po, lhsT=ones_sb[:],
                                             rhs=bias_sb[:, hsl],
                                             start=True, stop=False)
                        for dc in range(_DC):
                            nc.tensor.matmul(po, lhsT=xth[:, dc, bsl],
                                             rhs=kh_sb[:, dc, hsl],
                                             start=(not _BIAS_ON_PE and dc == 0),
                                             stop=False)
                            nc.tensor.matmul(po, lhsT=xth[:, dc, bsl],
                                             rhs=kl_sb[:, dc, hsl],
                                             start=False, stop=False)
                        for dc in range(_DC):
                            nc.tensor.matmul(po, lhsT=xtl[:, dc, bsl],
                                             rhs=kh_sb[:, dc, hsl],
                                             start=False, stop=(dc == _DC - 1))
                        if not _BIAS_ON_PE:
                            # s = 2*x.k - |k|^2, fused PSUM->SBUF move
                            nc.vector.tensor_sub(
                                out=s[:, hsl], in0=po, in1=k2_sb[:, hsl])
                    sc = ps if _BIAS_ON_PE else s
                    mx = stpool.tile([_P, 8], f32)
                    nc.vector.max(out=mx[:], in_=sc[:])
                    idx = stpool.tile([_P, 8], u32)
                    nc.vector.max_index(out=idx[:], in_max=mx[:], in_values=sc[:])

                    g = gpool.tile([_P, _D], f32)
                    nc.gpsimd.indirect_dma_start(
                        out=g[:],
                        out_offset=None,
                        in_=vals[:, :],
                        in_offset=IndirectOffsetOnAxis(ap=idx[:, :1], axis=0),
                    )
                    nc.sync.dma_start(out[bt * _BT:(bt + 1) * _BT, :], g[:])

    nc.compile()
    return nc


def _get_nc():
    global _cached
    if _cached is None:
        _cached = _build()
    return _cached


def _hi_lo(a):
    """Split fp32 array into bf16 hi + bf16 lo with hi + lo ~ a."""
    import ml_dtypes

    hi = a.astype(ml_dtypes.bfloat16)
    lo = (a - hi.astype(np.float32)).astype(ml_dtypes.bfloat16)
    return hi, lo


def _prepare_in_maps(x, keys, values):
    x = np.asarray(x, dtype=np.float32)
    keys = np.asarray(keys, dtype=np.float32)
    values = np.asarray(values, dtype=np.float32)

    import ml_dtypes

    k2T = np.ascontiguousarray((2.0 * keys).T)          # [512, 1024] f32
    kTh, kTl = _hi_lo(k2T)

    # 3-way bf16 split of -|k|^2, padded to 128 partition rows
    b64 = -np.einsum("kd,kd->k", keys.astype(np.float64),
                     keys.astype(np.float64))
    bh = b64.astype(ml_dtypes.bfloat16)
    bm = (b64 - bh.astype(np.float64)).astype(ml_dtypes.bfloat16)
    bl = (b64 - bh.astype(np.float64) - bm.astype(np.float64)).astype(
        ml_dtypes.bfloat16)
    biasp = np.zeros((_P, _K), dtype=ml_dtypes.bfloat16)
    biasp[0], biasp[1], biasp[2] = bh, bm, bl

    k2 = np.einsum("kd,kd->k", keys, keys).astype(np.float32)
    k2r = np.ascontiguousarray(np.broadcast_to(k2, (_P, _K)))

    in_maps = []
    for c in range(_NCORES):
        xs = np.ascontiguousarray(x[c * _BL:(c + 1) * _BL].T)  # [512, 8192]
        xh, xl = _hi_lo(xs)
        in_maps.append({"xTh": xh, "xTl": xl, "kTh": kTh, "kTl": kTl,
                        "biasp": biasp, "k2r": k2r, "vals": values})
    return in_maps


def kernel(x, keys, values):
    from concourse.bass_utils import run_bass_kernel_spmd

    nc = _get_nc()
    in_maps = _prepare_in_maps(x, keys, values)
    res = run_bass_kernel_spmd(nc, in_maps, core_ids=list(range(_NCORES)))
    return np.concatenate([r["out"] for r in res.results], axis=0)
